# revision 31
# baseline (speedup 1.0000x reference)
"""CentroidAttention Trainium2 kernel (8 NeuronCores, SPMD data-parallel over batch).

Reference computation (per problem):
    centers = segment_mean(features, labels, C=1000)       # [C, F]
    q = features @ Wq; k = centers @ Wk; v = centers @ Wv  # [B,A],[C,A],[C,A]
    P = softmax(q @ k.T / sqrt(A))                         # [B, C]
    attn = P @ v @ Wproj + bproj                           # [B, F]
    out = concat([features, attn], -1)                     # [B, 2F]

Sharding: batch B=16384 split 8 ways (2048 rows/core). Each core computes
partial segment sums+counts (as a one-hot matmul, transposed layout
sums.T [F, C]), AllReduce's them, then runs the attention pipeline on its
own batch shard. Weights are replicated.

Device layout choices (all matmuls are out = lhsT.T @ rhs, K on partitions):
  - sums.T [F, C]   <- lhsT = feat chunk [B,F-chunk], rhs = onehot [B, C]
  - feat.T [F, B]   <- PE transposes fused in the segsum pass (same lhsT)
  - q.T   [A, B]    <- lhsT = Wq [F, A], rhs = feat.T
  - kU.T  [A, C]    <- lhsT = Wk [F, A], rhs = sums.T      (unscaled by counts)
  - vU    [C, A]    <- lhsT = sums.T,    rhs = Wv          (unscaled)
  - S.T   [C, B]    <- lhsT = kU.T,      rhs = q.T
  - exp: ACT Exp with per-partition scale = attn_scale * recip_counts[c]
    (folds the centers division of the k-path into the softmax logits)
  - v = vU * recip_counts[c] applied on PSUM evict (folds the v-path division)
  - denom [1, B]    <- lhsT = ones[128,1], rhs = expS.T
  - attnU.T [A, B]  <- lhsT = v [C, A], rhs = expS.T   (unnormalized)
  - outU [B, F]     <- lhsT = attnU.T, rhs = Wproj, plus K=1 row
                       (lhsT=denom-row, rhs=bproj) so bias lands pre-normalized
  - out = outU * recip_denom[b]  (per-partition ACT scale on final evict)

Classes padded 1000 -> 1024 (zero one-hot columns); the padded expS.T rows are
memset to 0 so they contribute nothing to denom or PV.
"""

import numpy as np

import concourse.bass as bass
import concourse.bacc as bacc
import concourse.mybir as mybir
import concourse.tile as tile
from concourse.bass_utils import run_bass_kernel_spmd
from concourse.masks import make_identity

P = 128
B_LOCAL = 2048          # batch rows per core
F = 1024                # feature dim
A = 512                 # attention dim
C = 1000                # num classes
CP = 1024               # classes padded to a multiple of 512
NB = B_LOCAL // P       # 16 batch chunks
NF = F // P             # 8 feature chunks
NA = A // P             # 4 attn-dim chunks
NCC = CP // P           # 8 class chunks
NN = B_LOCAL // 512     # 4 moving-operand chunks over local batch
N_CORES = 8
SCALE = float(A) ** -0.5

F32 = mybir.dt.float32
BF16 = mybir.dt.bfloat16


def _emit(tc, collective=True):
    nc = tc.nc
    feat_dram = nc.dram_tensor("features", [B_LOCAL, F], F32, kind="ExternalInput")[:]
    lab_dram = nc.dram_tensor("labels_f32", [P, NB], F32, kind="ExternalInput")[:]
    wq_dram = nc.dram_tensor("Wq", [F, A], F32, kind="ExternalInput")[:]
    wk_dram = nc.dram_tensor("Wk", [F, A], F32, kind="ExternalInput")[:]
    wv_dram = nc.dram_tensor("Wv", [F, A], F32, kind="ExternalInput")[:]
    wp_dram = nc.dram_tensor("Wproj", [A, F], F32, kind="ExternalInput")[:]
    bp_dram = nc.dram_tensor("bproj", [1, F], F32, kind="ExternalInput")[:]
    out_dram = nc.dram_tensor("out", [B_LOCAL, F], F32, kind="ExternalOutput")[:]

    from contextlib import ExitStack

    with ExitStack() as ctx:
        consts = ctx.enter_context(tc.tile_pool(name="consts", bufs=1))
        stage = ctx.enter_context(tc.tile_pool(name="stage", bufs=1))
        featn_pool = ctx.enter_context(tc.tile_pool(name="featn", bufs=1))
        p1024 = ctx.enter_context(tc.tile_pool(name="p1024", bufs=1))
        t2048 = ctx.enter_context(tc.tile_pool(name="t2048", bufs=1))
        wpool = ctx.enter_context(tc.tile_pool(name="wpool", bufs=1))
        vpool = ctx.enter_context(tc.tile_pool(name="vpool", bufs=1))
        dram = ctx.enter_context(tc.tile_pool(name="dram", bufs=1, space="DRAM"))

        STAGE_BUFS = 4
        C1024_BUFS = 16
        T2048_BUFS = 12

        def stage_tile(name):
            return stage.tile([P, 1024], F32, name=name, tag="stage", bufs=STAGE_BUFS)

        def c1024_tile(name):
            return p1024.tile([P, CP], BF16, name=name, tag="c1024", bufs=C1024_BUFS)

        def t2048_tile(name):
            return t2048.tile([P, B_LOCAL], BF16, name=name, tag="t2048",
                              bufs=T2048_BUFS)

        # ---- constants ----
        identity = consts.tile([P, P], BF16, name="identity")
        make_identity(nc, identity)
        one1 = consts.tile([1, 1], F32, name="one1")
        nc.gpsimd.memset(one1, 1.0)
        ones_col = consts.tile([P, 1], BF16, name="ones_col")
        nc.gpsimd.memset(ones_col, 1.0)
        iota_g = consts.tile([P, CP], F32, name="iota_g")
        nc.gpsimd.iota(iota_g, pattern=[[1, CP]], base=0, channel_multiplier=0,
                       allow_small_or_imprecise_dtypes=True)
        # funnel iota + labels through DVE: the one-hot tensor_scalar
        # (pointer-scalar variant) only has a single sync-wait slot
        iota = consts.tile([P, CP], F32, name="iota")
        nc.vector.tensor_copy(iota, iota_g)
        labels_ld = consts.tile([P, NB], F32, name="labels_ld")
        nc.sync.dma_start(labels_ld, lab_dram)
        labels_sb = consts.tile([P, NB], F32, name="labels_sb")
        nc.vector.tensor_copy(labels_sb, labels_ld)

        # ---- collective bounce buffers (two halves so the reduce can start
        # while the second half of the segment sums is still computing) ----
        HALF = NF // 2 * P  # 512 rows of sums.T per collective
        bounce0_in = dram.tile([HALF + 1, CP], F32, name="bounce0_in")
        bounce0_out = dram.tile([HALF + 1, CP], F32, name="bounce0_out",
                                addr_space="Shared")
        bounce1_in = dram.tile([HALF, CP], F32, name="bounce1_in")
        bounce1_out = dram.tile([HALF, CP], F32, name="bounce1_out",
                                addr_space="Shared")

        # ---- phase 0: load features (cast bf16) and build one-hot ----
        feats = []
        for k in range(NB):
            st = stage_tile(f"fst{k}")
            nc.sync.dma_start(st, feat_dram[k * P:(k + 1) * P, :])
            fb = featn_pool.tile([P, F], BF16, name=f"featN{k}")
            nc.vector.tensor_copy(fb, st)
            feats.append(fb)
        onehots = []
        for k in range(NB):
            oh = c1024_tile(f"onehot{k}")
            nc.vector.tensor_scalar(oh, iota, labels_sb[:, k:k + 1], None,
                                    mybir.AluOpType.is_equal)
            onehots.append(oh)

        # ---- phase A: counts = ones.T @ onehot  -> bounce row F ----
        with tc.tile_pool(name="pcnt", bufs=1, space="PSUM") as pcnt:
            cps = pcnt.tile([1, CP], F32, name="counts_ps")
            for k in range(NB):
                for h in range(2):
                    nc.tensor.matmul(cps[:, h * 512:(h + 1) * 512],
                                     lhsT=ones_col,
                                     rhs=onehots[k][:, h * 512:(h + 1) * 512],
                                     start=(k == 0), stop=(k == NB - 1))
            cnt_sb = consts.tile([1, CP], F32, name="cnt_sb")
            nc.vector.tensor_copy(cnt_sb, cps)
            nc.sync.dma_start(bounce0_in[HALF:HALF + 1, :], cnt_sb)

        # ---- phase B: segment sums (transposed) + feat.T via fused PE
        # transpose. F-chunks processed in pairs so the PE has ~2x work per
        # arriving feature chunk during the initial DMA chase.
        featTs = [None] * NF
        with tc.tile_pool(name="pseg", bufs=1, space="PSUM") as pseg:
            for jp in range(0, NF, 2):
                sps_p, ftA_p, ftB_p = {}, {}, {}
                for j in (jp, jp + 1):
                    sps_p[j] = pseg.tile([P, CP], F32, name=f"sums{j}",
                                         tag="sums", bufs=2)
                    ftA_p[j] = pseg.tile([P, F], BF16, name=f"ftA{j}",
                                         tag="ftA", bufs=2)
                    ftB_p[j] = pseg.tile([P, F], BF16, name=f"ftB{j}",
                                         tag="ftB", bufs=2)
                for k in range(NB):
                    for j in (jp, jp + 1):
                        lhsT = feats[k][:, j * P:(j + 1) * P]
                        for h in range(2):
                            nc.tensor.matmul(
                                sps_p[j][:, h * 512:(h + 1) * 512],
                                lhsT=lhsT,
                                rhs=onehots[k][:, h * 512:(h + 1) * 512],
                                start=(k == 0), stop=(k == NB - 1))
                        ft = ftA_p[j] if k < 8 else ftB_p[j]
                        nc.tensor.transpose(ft[:, (k % 8) * P:(k % 8 + 1) * P],
                                            lhsT, identity)
                for j in (jp, jp + 1):
                    ftile = t2048_tile(f"featT{j}")
                    nc.vector.tensor_copy(ftile[:, 0:F], ftA_p[j])
                    nc.vector.tensor_copy(ftile[:, F:2 * F], ftB_p[j])
                    featTs[j] = ftile
                    sums_sb = stage_tile(f"sums_sb{j}")
                    nc.vector.tensor_copy(sums_sb, sps_p[j])
                    if j < NF // 2:
                        nc.sync.dma_start(bounce0_in[j * P:(j + 1) * P, :],
                                          sums_sb)
                    else:
                        jj = j - NF // 2
                        nc.sync.dma_start(bounce1_in[jj * P:(jj + 1) * P, :],
                                          sums_sb)
                if jp == NF // 2 - 2:
                    # first half (+counts) reduced while second half computes
                    if collective:
                        nc.gpsimd.collective_compute(
                            "AllReduce", mybir.AluOpType.add,
                            replica_groups=[list(range(N_CORES))],
                            ins=[bounce0_in.opt()], outs=[bounce0_out.opt()],
                        )
                    else:
                        nc.sync.dma_start(bounce0_out, bounce0_in)

        # ---- AllReduce second half ----
        if collective:
            nc.gpsimd.collective_compute(
                "AllReduce", mybir.AluOpType.add,
                replica_groups=[list(range(N_CORES))],
                ins=[bounce1_in.opt()], outs=[bounce1_out.opt()],
            )
        else:  # single-core timeline-sim variant: same traffic, no collective
            nc.sync.dma_start(bounce1_out, bounce1_in)

        # ---- weights: load + cast (overlaps the collective) ----
        wqb, wkb, wvb = [], [], []
        for nm, src, dst in (("wq", wq_dram, wqb), ("wk", wk_dram, wkb),
                             ("wv", wv_dram, wvb)):
            for j in range(NF):
                st = stage_tile(f"{nm}st{j}")
                nc.sync.dma_start(st[:, 0:A], src[j * P:(j + 1) * P, :])
                wb = wpool.tile([P, A], BF16, name=f"{nm}b{j}")
                nc.vector.tensor_copy(wb, st[:, 0:A])
                dst.append(wb)
        wpb = []
        for a in range(NA):
            st = stage_tile(f"wpst{a}")
            nc.sync.dma_start(st, wp_dram[a * P:(a + 1) * P, :])
            wb = wpool.tile([P, F], BF16, name=f"wpb{a}")
            nc.vector.tensor_copy(wb, st)
            wpb.append(wb)
        bst = stage_tile("bst")
        nc.sync.dma_start(bst[0:1, :], bp_dram)
        bprojb = wpool.tile([1, F], BF16, name="bprojb")
        nc.vector.tensor_copy(bprojb, bst[0:1, :])

        # ---- q.T = Wq.T @ feat.T (PE busy during the collective) ----
        qTs = []
        with tc.tile_pool(name="pq", bufs=1, space="PSUM") as pq:
            for a in range(NA):
                qt = t2048_tile(f"qT{a}")
                for nh in range(2):
                    qps = pq.tile([P, F], F32, name=f"qps{a}_{nh}",
                                  tag="q", bufs=4)
                    for j in range(NF):
                        for n in range(2):
                            nc.tensor.matmul(qps[:, n * 512:(n + 1) * 512],
                                             lhsT=wqb[j][:, a * P:(a + 1) * P],
                                             rhs=featTs[j][:, (nh * 2 + n) * 512:
                                                           (nh * 2 + n + 1) * 512],
                                             start=(j == 0), stop=(j == NF - 1))
                    nc.scalar.copy(qt[:, nh * F:(nh + 1) * F], qps)
                qTs.append(qt)

        # ---- read back reduced sums + counts ----
        sumsb = []
        for j in range(NF):
            st = stage_tile(f"sst{j}")
            if j < NF // 2:
                nc.sync.dma_start(st, bounce0_out[j * P:(j + 1) * P, :])
            else:
                jj = j - NF // 2
                nc.sync.dma_start(st, bounce1_out[jj * P:(jj + 1) * P, :])
            sb = c1024_tile(f"sumsb{j}")
            nc.vector.tensor_copy(sb, st)
            sumsb.append(sb)
        counts_sb = consts.tile([1, CP], F32, name="counts_sb")
        nc.sync.dma_start(counts_sb, bounce0_out[HALF:HALF + 1, :])

        kTs, vbs = [], []
        with tc.tile_pool(name="pkv", bufs=1, space="PSUM") as pkv:
            # recip_counts in [C-chunk(partition), chunk-idx] layout
            cpsT = pkv.tile([P, NCC], F32, name="cntT")
            for c in range(NCC):
                nc.tensor.transpose(cpsT[:, c:c + 1],
                                    counts_sb[:, c * P:(c + 1) * P], one1)
            cnt_m = consts.tile([P, NCC], F32, name="cnt_m")
            nc.vector.tensor_scalar_max(cnt_m, cpsT, 1.0)
            recip_cols = consts.tile([P, NCC], F32, name="recip_cols")
            nc.vector.reciprocal(recip_cols, cnt_m)
            exp_scale = consts.tile([P, NCC], F32, name="exp_scale")
            nc.vector.tensor_scalar_mul(exp_scale, recip_cols, SCALE)

            # kU.T [A, C] ; counts division folded into the exp scale later
            for a in range(NA):
                kps = pkv.tile([P, CP], F32, name=f"kps{a}", tag="k", bufs=2)
                for j in range(NF):
                    for h in range(2):
                        nc.tensor.matmul(kps[:, h * 512:(h + 1) * 512],
                                         lhsT=wkb[j][:, a * P:(a + 1) * P],
                                         rhs=sumsb[j][:, h * 512:(h + 1) * 512],
                                         start=(j == 0), stop=(j == NF - 1))
                kt = c1024_tile(f"kT{a}")
                nc.scalar.copy(kt, kps)
                kTs.append(kt)

            # v [C, A] = (sums.T).T @ Wv, scaled by recip_counts on evict
            for c in range(NCC):
                vps = pkv.tile([P, A], F32, name=f"vps{c}", tag="v", bufs=2)
                for j in range(NF):
                    nc.tensor.matmul(vps,
                                     lhsT=sumsb[j][:, c * P:(c + 1) * P],
                                     rhs=wvb[j],
                                     start=(j == 0), stop=(j == NF - 1))
                vb = vpool.tile([P, A], BF16, name=f"vb{c}")
                nc.scalar.activation(vb, vps,
                                     mybir.ActivationFunctionType.Copy,
                                     bias=0.0, scale=recip_cols[:, c:c + 1])
                vbs.append(vb)

        # ---- S.T [C, B] and exp (centers division folded into scale) ----
        expSTs = []
        with tc.tile_pool(name="pst", bufs=1, space="PSUM") as pst:
            for c in range(NCC):
                est = t2048_tile(f"expST{c}")
                rows = (C - c * P) if c == NCC - 1 else P
                if rows < P:
                    # zero the padded class rows; exp overwrites the valid ones
                    nc.vector.memset(est, 0.0)
                for nh in range(2):
                    sps = pst.tile([P, F], F32, name=f"stps{c}_{nh}",
                                   tag="st", bufs=4)
                    for a in range(NA):
                        for n in range(2):
                            nc.tensor.matmul(
                                sps[:, n * 512:(n + 1) * 512],
                                lhsT=kTs[a][:, c * P:(c + 1) * P],
                                rhs=qTs[a][:, (nh * 2 + n) * 512:
                                            (nh * 2 + n + 1) * 512],
                                start=(a == 0), stop=(a == NA - 1))
                    nc.scalar.activation(est[0:rows, nh * F:(nh + 1) * F],
                                         sps[0:rows, :],
                                         mybir.ActivationFunctionType.Exp,
                                         bias=0.0,
                                         scale=exp_scale[0:rows, c:c + 1])
                expSTs.append(est)

        # ---- softmax denominator [1, B] + its reciprocal transposed ----
        recipD_cols = consts.tile([P, NB], F32, name="recipD_cols")
        den_b = consts.tile([1, B_LOCAL], BF16, name="den_b")
        with tc.tile_pool(name="pden", bufs=1, space="PSUM") as pden:
            dps = pden.tile([1, B_LOCAL], F32, name="dps")
            for c in range(NCC):
                for n in range(NN):
                    nc.tensor.matmul(dps[:, n * 512:(n + 1) * 512],
                                     lhsT=ones_col,
                                     rhs=expSTs[c][:, n * 512:(n + 1) * 512],
                                     start=(c == 0), stop=(c == NCC - 1))
            recipD = consts.tile([1, B_LOCAL], F32, name="recipD")
            nc.vector.reciprocal(recipD, dps)
            nc.vector.tensor_copy(den_b, dps)
            rdps = pden.tile([P, NB], F32, name="rdps")
            for t in range(NB):
                nc.tensor.transpose(rdps[:, t:t + 1],
                                    recipD[:, t * P:(t + 1) * P], one1)
            nc.vector.tensor_copy(recipD_cols, rdps)

        # ---- attnU.T [A, B] = v.T @ expS.T (unnormalized) ----
        attnTs = []
        with tc.tile_pool(name="ppv", bufs=1, space="PSUM") as ppv:
            for a in range(NA):
                at = t2048_tile(f"attnT{a}")
                for nh in range(2):
                    aps = ppv.tile([P, F], F32, name=f"aps{a}_{nh}",
                                   tag="av", bufs=4)
                    for c in range(NCC):
                        for n in range(2):
                            nc.tensor.matmul(
                                aps[:, n * 512:(n + 1) * 512],
                                lhsT=vbs[c][:, a * P:(a + 1) * P],
                                rhs=expSTs[c][:, (nh * 2 + n) * 512:
                                              (nh * 2 + n + 1) * 512],
                                start=(c == 0), stop=(c == NCC - 1))
                    evict = nc.vector.tensor_copy if nh == 0 else nc.scalar.copy
                    evict(at[:, nh * F:(nh + 1) * F], aps)
                attnTs.append(at)

        # ---- out = (attnU.T.T @ Wproj + denom*bproj) * recip_denom ----
        with tc.tile_pool(name="po", bufs=1, space="PSUM") as po:
            for t in range(NB):
                ops = po.tile([P, F], F32, name=f"ops{t}", tag="o", bufs=3)
                for a in range(NA):
                    for h in range(2):
                        nc.tensor.matmul(ops[:, h * 512:(h + 1) * 512],
                                         lhsT=attnTs[a][:, t * P:(t + 1) * P],
                                         rhs=wpb[a][:, h * 512:(h + 1) * 512],
                                         start=(a == 0), stop=False)
                for h in range(2):
                    nc.tensor.matmul(ops[:, h * 512:(h + 1) * 512],
                                     lhsT=den_b[:, t * P:(t + 1) * P],
                                     rhs=bprojb[:, h * 512:(h + 1) * 512],
                                     start=False, stop=True)
                osb = stage_tile(f"osb{t}")
                nc.scalar.activation(osb, ops,
                                     mybir.ActivationFunctionType.Copy,
                                     bias=0.0, scale=recipD_cols[:, t:t + 1])
                nc.sync.dma_start(out_dram[t * P:(t + 1) * P, :], osb)


_BUILT = {}


def _get_nc(collective=True):
    if collective not in _BUILT:
        nc = bacc.Bacc("TRN2", target_bir_lowering=False, debug=False,
                       num_devices=N_CORES)
        with tile.TileContext(nc) as tc:
            _emit(tc, collective=collective)
        nc.compile()
        _BUILT[collective] = nc
    return _BUILT[collective]


def _make_in_maps(inputs):
    features = np.ascontiguousarray(np.asarray(inputs["features"],
                                               dtype=np.float32))
    labels = np.ascontiguousarray(np.asarray(inputs["labels"])).astype(np.int64)
    Wq = np.ascontiguousarray(np.asarray(inputs["Wq"], dtype=np.float32))
    Wk = np.ascontiguousarray(np.asarray(inputs["Wk"], dtype=np.float32))
    Wv = np.ascontiguousarray(np.asarray(inputs["Wv"], dtype=np.float32))
    Wproj = np.ascontiguousarray(np.asarray(inputs["Wproj"], dtype=np.float32))
    bproj = np.ascontiguousarray(
        np.asarray(inputs["bproj"], dtype=np.float32)).reshape(1, F)

    in_maps = []
    for cix in range(N_CORES):
        fl = features[cix * B_LOCAL:(cix + 1) * B_LOCAL]
        ll = labels[cix * B_LOCAL:(cix + 1) * B_LOCAL]
        lab2d = np.ascontiguousarray(
            ll.astype(np.float32).reshape(NB, P).T)
        in_maps.append({
            "features": fl,
            "labels_f32": lab2d,
            "Wq": Wq, "Wk": Wk, "Wv": Wv, "Wproj": Wproj, "bproj": bproj,
        })
    return in_maps


def _assemble(inputs, results):
    features = np.asarray(inputs["features"], dtype=np.float32)
    out = np.empty((N_CORES * B_LOCAL, 2 * F), np.float32)
    out[:, :F] = features
    for cix in range(N_CORES):
        out[cix * B_LOCAL:(cix + 1) * B_LOCAL, F:] = results[cix]["out"]
    return out


def _run(inputs, **run_kwargs):
    nc = _get_nc()
    in_maps = _make_in_maps(inputs)
    res = run_bass_kernel_spmd(nc, in_maps, list(range(N_CORES)), **run_kwargs)
    return _assemble(inputs, res.results), res


def kernel(**inputs):
    out, _ = _run(inputs)
    return out


# revision 44
# speedup vs baseline: 2.9631x; 2.9631x over previous
"""CentroidAttention Trainium2 kernel (8 NeuronCores, SPMD data-parallel over batch).

Reference computation (per problem):
    centers = segment_mean(features, labels, C=1000)       # [C, F]
    q = features @ Wq; k = centers @ Wk; v = centers @ Wv  # [B,A],[C,A],[C,A]
    P = softmax(q @ k.T / sqrt(A))                         # [B, C]
    attn = P @ v @ Wproj + bproj                           # [B, F]
    out = concat([features, attn], -1)                     # [B, 2F]

Sharding: batch B=16384 split 8 ways (2048 rows/core). Each core computes
partial segment sums+counts (as a one-hot matmul, transposed layout
sums.T [F, C]), AllReduce's them, then runs the attention pipeline on its
own batch shard. Weights are replicated.

Device layout choices (all matmuls are out = lhsT.T @ rhs, K on partitions):
  - sums.T [F, C]   <- lhsT = feat chunk [B,F-chunk], rhs = onehot [B, C]
  - feat.T [F, B]   <- PE transposes fused in the segsum pass (same lhsT)
  - q.T   [A, B]    <- lhsT = Wq [F, A], rhs = feat.T
  - kU.T  [A, C]    <- lhsT = Wk [F, A], rhs = sums.T      (unscaled by counts)
  - vU    [C, A]    <- lhsT = sums.T,    rhs = Wv          (unscaled)
  - S.T   [C, B]    <- lhsT = kU.T,      rhs = q.T
  - exp: ACT Exp with per-partition scale = attn_scale * recip_counts[c]
    (folds the centers division of the k-path into the softmax logits)
  - v = vU * recip_counts[c] applied on PSUM evict (folds the v-path division)
  - denom [1, B]    <- lhsT = ones[128,1], rhs = expS.T
  - attnU.T [A, B]  <- lhsT = v [C, A], rhs = expS.T   (unnormalized)
  - outU [B, F]     <- lhsT = attnU.T, rhs = Wproj, plus K=1 row
                       (lhsT=denom-row, rhs=bproj) so bias lands pre-normalized
  - out = outU * recip_denom[b]  (per-partition ACT scale on final evict)

Classes padded 1000 -> 1024 (zero one-hot columns); the padded expS.T rows are
memset to 0 so they contribute nothing to denom or PV.
"""

import numpy as np

import concourse.bass as bass
import concourse.bacc as bacc
import concourse.mybir as mybir
import concourse.tile as tile
from concourse.bass_utils import run_bass_kernel_spmd
from concourse.masks import make_identity

P = 128
B_LOCAL = 2048          # batch rows per core
F = 1024                # feature dim
A = 512                 # attention dim
C = 1000                # num classes
CP = 1024               # classes padded to a multiple of 512
NB = B_LOCAL // P       # 16 batch chunks
NF = F // P             # 8 feature chunks
NA = A // P             # 4 attn-dim chunks
NCC = CP // P           # 8 class chunks
NN = B_LOCAL // 512     # 4 moving-operand chunks over local batch
N_CORES = 8
SCALE = float(A) ** -0.5

F32 = mybir.dt.float32
BF16 = mybir.dt.bfloat16
F16 = mybir.dt.float16


def _emit(tc, collective=True, io=None):
    nc = tc.nc
    if io is None:
        io = _declare_io(nc)
    (feat_dram, lab_dram, wq_dram, wk_dram, wv_dram, wp_dram, bp_dram,
     out_dram) = io

    from contextlib import ExitStack

    with ExitStack() as ctx:
        consts = ctx.enter_context(tc.tile_pool(name="consts", bufs=1))
        stage = ctx.enter_context(tc.tile_pool(name="stage", bufs=1))
        featn_pool = ctx.enter_context(tc.tile_pool(name="featn", bufs=1))
        p1024 = ctx.enter_context(tc.tile_pool(name="p1024", bufs=1))
        t2048 = ctx.enter_context(tc.tile_pool(name="t2048", bufs=1))
        wpool = ctx.enter_context(tc.tile_pool(name="wpool", bufs=1))
        vpool = ctx.enter_context(tc.tile_pool(name="vpool", bufs=1))
        dram = ctx.enter_context(tc.tile_pool(name="dram", bufs=1, space="DRAM"))

        STAGE_BUFS = 4
        C1024_BUFS = 16
        T2048_BUFS = 12
        pf16 = ctx.enter_context(tc.tile_pool(name="pf16", bufs=1))

        def stage_tile(name):
            return stage.tile([P, 1024], F32, name=name, tag="stage", bufs=STAGE_BUFS)

        def c1024_tile(name):
            return p1024.tile([P, CP], BF16, name=name, tag="c1024", bufs=C1024_BUFS)

        def t2048_tile(name):
            return t2048.tile([P, B_LOCAL], BF16, name=name, tag="t2048",
                              bufs=T2048_BUFS)

        # ---- constants ----
        identity = consts.tile([P, P], BF16, name="identity")
        make_identity(nc, identity)
        one1 = consts.tile([1, 1], F32, name="one1")
        nc.gpsimd.memset(one1, 1.0)
        ones_col = consts.tile([P, 1], BF16, name="ones_col")
        nc.gpsimd.memset(ones_col, 1.0)
        iota_g = consts.tile([P, CP], F32, name="iota_g")
        nc.gpsimd.iota(iota_g, pattern=[[1, CP]], base=0, channel_multiplier=0,
                       allow_small_or_imprecise_dtypes=True)
        # funnel iota + labels through DVE: the one-hot tensor_scalar
        # (pointer-scalar variant) only has a single sync-wait slot
        iota = consts.tile([P, CP], F32, name="iota")
        nc.vector.tensor_copy(iota, iota_g)
        labels_ld = consts.tile([P, NB], F32, name="labels_ld")
        nc.sync.dma_start(labels_ld, lab_dram)
        labels_sb = consts.tile([P, NB], F32, name="labels_sb")
        nc.vector.tensor_copy(labels_sb, labels_ld)
        # warm the ACT Exp table during the load phase so the table DMA
        # doesn't land in the middle of the softmax
        exp_warm = consts.tile([P, 1], F32, name="exp_warm")
        nc.scalar.activation(exp_warm, labels_sb[:, 0:1],
                             mybir.ActivationFunctionType.Exp,
                             bias=0.0, scale=0.0)

        # ---- collective bounce buffers. counts ride their own tiny f32
        # reduce launched during the segsum; the two sums halves go as fp16
        # (sums are already bf16-feature-derived, fp16 wire rounding is
        # negligible) so each collective moves 1 MB instead of 2.1 ----
        HALF = NF // 2 * P  # 512 rows of sums.T per collective
        bcnt_in = dram.tile([1, CP], F32, name="bcnt_in")
        bcnt_out = dram.tile([1, CP], F32, name="bcnt_out",
                             addr_space="Shared")
        bounce0_in = dram.tile([HALF, CP], F16, name="bounce0_in")
        bounce0_out = dram.tile([HALF, CP], F16, name="bounce0_out",
                                addr_space="Shared")
        bounce1_in = dram.tile([HALF, CP], F16, name="bounce1_in")
        bounce1_out = dram.tile([HALF, CP], F16, name="bounce1_out",
                                addr_space="Shared")

        # ---- phase 0: load features (cast bf16) and build one-hot ----
        feats = []
        for k in range(NB):
            st = stage_tile(f"fst{k}")
            nc.sync.dma_start(st, feat_dram[k * P:(k + 1) * P, :])
            fb = featn_pool.tile([P, F], BF16, name=f"featN{k}")
            # ACT does the cast: DVE is saturated building one-hots during
            # the feature-load chase
            nc.scalar.copy(fb, st)
            feats.append(fb)
        onehots = []
        for k in range(NB):
            oh = c1024_tile(f"onehot{k}")
            nc.vector.tensor_scalar(oh, iota, labels_sb[:, k:k + 1], None,
                                    mybir.AluOpType.is_equal)
            onehots.append(oh)

        # ---- phase A: counts = ones.T @ onehot  -> bounce row F ----
        with tc.tile_pool(name="pcnt", bufs=1, space="PSUM") as pcnt:
            cps = pcnt.tile([1, CP], F32, name="counts_ps")
            for k in range(NB):
                for h in range(2):
                    nc.tensor.matmul(cps[:, h * 512:(h + 1) * 512],
                                     lhsT=ones_col,
                                     rhs=onehots[k][:, h * 512:(h + 1) * 512],
                                     start=(k == 0), stop=(k == NB - 1))
            cnt_sb = consts.tile([1, CP], F32, name="cnt_sb")
            nc.vector.tensor_copy(cnt_sb, cps)
            nc.sync.dma_start(bcnt_in, cnt_sb)
        # counts reduce in flight while the segment sums still compute
        if collective:
            nc.gpsimd.collective_compute(
                "AllReduce", mybir.AluOpType.add,
                replica_groups=[list(range(N_CORES))],
                ins=[bcnt_in.opt()], outs=[bcnt_out.opt()],
            )
        else:
            nc.sync.dma_start(bcnt_out, bcnt_in)

        # ---- phase B: segment sums (transposed) + feat.T via fused PE
        # transpose. F-chunks processed in pairs so the PE has ~2x work per
        # arriving feature chunk during the initial DMA chase.
        featTs = [None] * NF
        with tc.tile_pool(name="pseg", bufs=1, space="PSUM") as pseg:
            for jp in range(0, NF, 2):
                sps_p, ftA_p, ftB_p = {}, {}, {}
                for j in (jp, jp + 1):
                    sps_p[j] = pseg.tile([P, CP], F32, name=f"sums{j}",
                                         tag="sums", bufs=2)
                    ftA_p[j] = pseg.tile([P, F], BF16, name=f"ftA{j}",
                                         tag="ftA", bufs=2)
                    ftB_p[j] = pseg.tile([P, F], BF16, name=f"ftB{j}",
                                         tag="ftB", bufs=2)
                for k in range(NB):
                    for j in (jp, jp + 1):
                        lhsT = feats[k][:, j * P:(j + 1) * P]
                        for h in range(2):
                            nc.tensor.matmul(
                                sps_p[j][:, h * 512:(h + 1) * 512],
                                lhsT=lhsT,
                                rhs=onehots[k][:, h * 512:(h + 1) * 512],
                                start=(k == 0), stop=(k == NB - 1))
                        ft = ftA_p[j] if k < 8 else ftB_p[j]
                        nc.tensor.transpose(ft[:, (k % 8) * P:(k % 8 + 1) * P],
                                            lhsT, identity)
                for j in (jp, jp + 1):
                    ftile = t2048_tile(f"featT{j}")
                    nc.vector.tensor_copy(ftile[:, 0:F], ftA_p[j])
                    nc.vector.tensor_copy(ftile[:, F:2 * F], ftB_p[j])
                    featTs[j] = ftile
                    sums_sb = pf16.tile([P, CP], F16, name=f"sums_f16_{j}",
                                        tag="sf16", bufs=3)
                    nc.vector.tensor_copy(sums_sb, sps_p[j])
                    if j < NF // 2:
                        nc.sync.dma_start(bounce0_in[j * P:(j + 1) * P, :],
                                          sums_sb)
                    else:
                        jj = j - NF // 2
                        nc.sync.dma_start(bounce1_in[jj * P:(jj + 1) * P, :],
                                          sums_sb)
                if jp == NF // 2 - 2:
                    # first half (+counts) reduced while second half computes
                    if collective:
                        nc.gpsimd.collective_compute(
                            "AllReduce", mybir.AluOpType.add,
                            replica_groups=[list(range(N_CORES))],
                            ins=[bounce0_in.opt()], outs=[bounce0_out.opt()],
                        )
                    else:
                        nc.sync.dma_start(bounce0_out, bounce0_in)

        # ---- AllReduce second half ----
        if collective:
            nc.gpsimd.collective_compute(
                "AllReduce", mybir.AluOpType.add,
                replica_groups=[list(range(N_CORES))],
                ins=[bounce1_in.opt()], outs=[bounce1_out.opt()],
            )
        else:  # single-core timeline-sim variant: same traffic, no collective
            nc.sync.dma_start(bounce1_out, bounce1_in)

        # ---- weights: load + cast (overlaps the collective) ----
        wqb, wkb, wvb = [], [], []
        for nm, src, dst in (("wq", wq_dram, wqb), ("wk", wk_dram, wkb),
                             ("wv", wv_dram, wvb)):
            for j in range(NF):
                st = stage_tile(f"{nm}st{j}")
                nc.sync.dma_start(st[:, 0:A], src[j * P:(j + 1) * P, :])
                wb = wpool.tile([P, A], BF16, name=f"{nm}b{j}")
                nc.vector.tensor_copy(wb, st[:, 0:A])
                dst.append(wb)
        wpb = []
        for a in range(NA):
            st = stage_tile(f"wpst{a}")
            nc.sync.dma_start(st, wp_dram[a * P:(a + 1) * P, :])
            wb = wpool.tile([P, F], BF16, name=f"wpb{a}")
            nc.vector.tensor_copy(wb, st)
            wpb.append(wb)
        bst = stage_tile("bst")
        nc.sync.dma_start(bst[0:1, :], bp_dram)
        bprojb = wpool.tile([1, F], BF16, name="bprojb")
        nc.vector.tensor_copy(bprojb, bst[0:1, :])

        # ---- q.T = Wq.T @ feat.T (PE busy during the collective) ----
        qTs = []
        with tc.tile_pool(name="pq", bufs=1, space="PSUM") as pq:
            for a in range(NA):
                qt = t2048_tile(f"qT{a}")
                for nh in range(2):
                    qps = pq.tile([P, F], F32, name=f"qps{a}_{nh}",
                                  tag="q", bufs=4)
                    for j in range(NF):
                        for n in range(2):
                            nc.tensor.matmul(qps[:, n * 512:(n + 1) * 512],
                                             lhsT=wqb[j][:, a * P:(a + 1) * P],
                                             rhs=featTs[j][:, (nh * 2 + n) * 512:
                                                           (nh * 2 + n + 1) * 512],
                                             start=(j == 0), stop=(j == NF - 1))
                    nc.scalar.copy(qt[:, nh * F:(nh + 1) * F], qps)
                qTs.append(qt)

        # ---- read back reduced sums + counts ----
        sumsb = []
        for j in range(NF):
            st = stage.tile([P, CP], F16, name=f"sst{j}", tag="stage",
                            bufs=STAGE_BUFS)
            if j < NF // 2:
                nc.sync.dma_start(st, bounce0_out[j * P:(j + 1) * P, :])
            else:
                jj = j - NF // 2
                nc.sync.dma_start(st, bounce1_out[jj * P:(jj + 1) * P, :])
            sb = c1024_tile(f"sumsb{j}")
            nc.vector.tensor_copy(sb, st)
            sumsb.append(sb)
        counts_sb = consts.tile([1, CP], F32, name="counts_sb")
        nc.sync.dma_start(counts_sb, bcnt_out)

        kTs, vbs = [], []
        with tc.tile_pool(name="pkv", bufs=1, space="PSUM") as pkv:
            # recip_counts in [C-chunk(partition), chunk-idx] layout
            cpsT = pkv.tile([P, NCC], F32, name="cntT")
            for c in range(NCC):
                nc.tensor.transpose(cpsT[:, c:c + 1],
                                    counts_sb[:, c * P:(c + 1) * P], one1)
            cnt_m = consts.tile([P, NCC], F32, name="cnt_m")
            nc.vector.tensor_scalar_max(cnt_m, cpsT, 1.0)
            recip_cols = consts.tile([P, NCC], F32, name="recip_cols")
            nc.vector.reciprocal(recip_cols, cnt_m)
            exp_scale = consts.tile([P, NCC], F32, name="exp_scale")
            nc.vector.tensor_scalar_mul(exp_scale, recip_cols, SCALE)

            # kU.T [A, C] ; counts division folded into the exp scale later
            for a in range(NA):
                kps = pkv.tile([P, CP], F32, name=f"kps{a}", tag="k", bufs=2)
                for j in range(NF):
                    for h in range(2):
                        nc.tensor.matmul(kps[:, h * 512:(h + 1) * 512],
                                         lhsT=wkb[j][:, a * P:(a + 1) * P],
                                         rhs=sumsb[j][:, h * 512:(h + 1) * 512],
                                         start=(j == 0), stop=(j == NF - 1))
                kt = c1024_tile(f"kT{a}")
                nc.scalar.copy(kt, kps)
                kTs.append(kt)

            # v [C, A] = (sums.T).T @ Wv, scaled by recip_counts on evict
            for c in range(NCC):
                vps = pkv.tile([P, A], F32, name=f"vps{c}", tag="v", bufs=2)
                for j in range(NF):
                    nc.tensor.matmul(vps,
                                     lhsT=sumsb[j][:, c * P:(c + 1) * P],
                                     rhs=wvb[j],
                                     start=(j == 0), stop=(j == NF - 1))
                vb = vpool.tile([P, A], BF16, name=f"vb{c}")
                nc.scalar.activation(vb, vps,
                                     mybir.ActivationFunctionType.Copy,
                                     bias=0.0, scale=recip_cols[:, c:c + 1])
                vbs.append(vb)

        # ---- S.T [C, B] and exp (centers division folded into scale) ----
        expSTs = []
        with tc.tile_pool(name="pst", bufs=1, space="PSUM") as pst:
            for c in range(NCC):
                est = t2048_tile(f"expST{c}")
                rows = (C - c * P) if c == NCC - 1 else P
                if rows < P:
                    # zero the padded class rows; exp overwrites the valid ones
                    nc.vector.memset(est, 0.0)
                for nh in range(2):
                    sps = pst.tile([P, F], F32, name=f"stps{c}_{nh}",
                                   tag="st", bufs=4)
                    for a in range(NA):
                        for n in range(2):
                            nc.tensor.matmul(
                                sps[:, n * 512:(n + 1) * 512],
                                lhsT=kTs[a][:, c * P:(c + 1) * P],
                                rhs=qTs[a][:, (nh * 2 + n) * 512:
                                            (nh * 2 + n + 1) * 512],
                                start=(a == 0), stop=(a == NA - 1))
                    nc.scalar.activation(est[0:rows, nh * F:(nh + 1) * F],
                                         sps[0:rows, :],
                                         mybir.ActivationFunctionType.Exp,
                                         bias=0.0,
                                         scale=exp_scale[0:rows, c:c + 1])
                expSTs.append(est)

        # ---- attnU.T [A, B] = v.T @ expS.T (unnormalized), with the softmax
        # denominator accumulated in the same phase (a==0 pass) ----
        recipD_cols = consts.tile([P, NB], F32, name="recipD_cols")
        den_b = consts.tile([1, B_LOCAL], BF16, name="den_b")
        recipD = consts.tile([1, B_LOCAL], F32, name="recipD")
        attnTs = []
        with tc.tile_pool(name="ppv", bufs=1, space="PSUM") as ppv:
            dps = ppv.tile([1, B_LOCAL], F32, name="dps")
            for a in range(NA):
                at = t2048_tile(f"attnT{a}")
                for nh in range(2):
                    aps = ppv.tile([P, F], F32, name=f"aps{a}_{nh}",
                                   tag="av", bufs=2)
                    for c in range(NCC):
                        for n in range(2):
                            nc.tensor.matmul(
                                aps[:, n * 512:(n + 1) * 512],
                                lhsT=vbs[c][:, a * P:(a + 1) * P],
                                rhs=expSTs[c][:, (nh * 2 + n) * 512:
                                              (nh * 2 + n + 1) * 512],
                                start=(c == 0), stop=(c == NCC - 1))
                        if a == 0:
                            for n in range(2):
                                nc.tensor.matmul(
                                    dps[:, (nh * 2 + n) * 512:
                                        (nh * 2 + n + 1) * 512],
                                    lhsT=ones_col,
                                    rhs=expSTs[c][:, (nh * 2 + n) * 512:
                                                  (nh * 2 + n + 1) * 512],
                                    start=(c == 0), stop=(c == NCC - 1))
                    evict = nc.vector.tensor_copy if nh == 0 else nc.scalar.copy
                    evict(at[:, nh * F:(nh + 1) * F], aps)
                if a == 0:
                    nc.vector.reciprocal(recipD, dps)
                    nc.vector.tensor_copy(den_b, dps)
                attnTs.append(at)

        # ---- out = (attnU.T.T @ Wproj + denom*bproj) * recip_denom ----
        with tc.tile_pool(name="po", bufs=1, space="PSUM") as po:
            rdps = po.tile([P, NB], F32, name="rdps")
            for t in range(NB):
                nc.tensor.transpose(rdps[:, t:t + 1],
                                    recipD[:, t * P:(t + 1) * P], one1)
            nc.vector.tensor_copy(recipD_cols, rdps)
            for t in range(NB):
                ops = po.tile([P, F], F32, name=f"ops{t}", tag="o", bufs=3)
                for a in range(NA):
                    for h in range(2):
                        nc.tensor.matmul(ops[:, h * 512:(h + 1) * 512],
                                         lhsT=attnTs[a][:, t * P:(t + 1) * P],
                                         rhs=wpb[a][:, h * 512:(h + 1) * 512],
                                         start=(a == 0), stop=False)
                for h in range(2):
                    nc.tensor.matmul(ops[:, h * 512:(h + 1) * 512],
                                     lhsT=den_b[:, t * P:(t + 1) * P],
                                     rhs=bprojb[:, h * 512:(h + 1) * 512],
                                     start=False, stop=True)
                osb = stage_tile(f"osb{t}")
                nc.scalar.activation(osb, ops,
                                     mybir.ActivationFunctionType.Copy,
                                     bias=0.0, scale=recipD_cols[:, t:t + 1])
                nc.sync.dma_start(out_dram[t * P:(t + 1) * P, :], osb)


def _declare_io(nc):
    return (
        nc.dram_tensor("features", [B_LOCAL, F], F32, kind="ExternalInput")[:],
        nc.dram_tensor("labels_f32", [P, NB], F32, kind="ExternalInput")[:],
        nc.dram_tensor("Wq", [F, A], F32, kind="ExternalInput")[:],
        nc.dram_tensor("Wk", [F, A], F32, kind="ExternalInput")[:],
        nc.dram_tensor("Wv", [F, A], F32, kind="ExternalInput")[:],
        nc.dram_tensor("Wproj", [A, F], F32, kind="ExternalInput")[:],
        nc.dram_tensor("bproj", [1, F], F32, kind="ExternalInput")[:],
        nc.dram_tensor("out", [B_LOCAL, F], F32, kind="ExternalOutput")[:],
    )


_BUILT = {}


def _get_nc(collective=True, reps=1):
    key = (collective, reps)
    if key not in _BUILT:
        nc = bacc.Bacc("TRN2", target_bir_lowering=False, debug=False,
                       num_devices=N_CORES)
        with tile.TileContext(nc) as tc:
            io = _declare_io(nc)
            for r in range(reps):
                if r:
                    tc.strict_bb_all_engine_barrier()
                _emit(tc, collective=collective, io=io)
        nc.compile()
        _BUILT[key] = nc
    return _BUILT[key]


def _make_in_maps(inputs):
    features = np.ascontiguousarray(np.asarray(inputs["features"],
                                               dtype=np.float32))
    labels = np.ascontiguousarray(np.asarray(inputs["labels"])).astype(np.int64)
    Wq = np.ascontiguousarray(np.asarray(inputs["Wq"], dtype=np.float32))
    Wk = np.ascontiguousarray(np.asarray(inputs["Wk"], dtype=np.float32))
    Wv = np.ascontiguousarray(np.asarray(inputs["Wv"], dtype=np.float32))
    Wproj = np.ascontiguousarray(np.asarray(inputs["Wproj"], dtype=np.float32))
    bproj = np.ascontiguousarray(
        np.asarray(inputs["bproj"], dtype=np.float32)).reshape(1, F)

    in_maps = []
    for cix in range(N_CORES):
        fl = features[cix * B_LOCAL:(cix + 1) * B_LOCAL]
        ll = labels[cix * B_LOCAL:(cix + 1) * B_LOCAL]
        lab2d = np.ascontiguousarray(
            ll.astype(np.float32).reshape(NB, P).T)
        in_maps.append({
            "features": fl,
            "labels_f32": lab2d,
            "Wq": Wq, "Wk": Wk, "Wv": Wv, "Wproj": Wproj, "bproj": bproj,
        })
    return in_maps


def _assemble(inputs, results):
    features = np.asarray(inputs["features"], dtype=np.float32)
    out = np.empty((N_CORES * B_LOCAL, 2 * F), np.float32)
    out[:, :F] = features
    for cix in range(N_CORES):
        out[cix * B_LOCAL:(cix + 1) * B_LOCAL, F:] = results[cix]["out"]
    return out


def _run(inputs, **run_kwargs):
    nc = _get_nc()
    in_maps = _make_in_maps(inputs)
    res = run_bass_kernel_spmd(nc, in_maps, list(range(N_CORES)), **run_kwargs)
    return _assemble(inputs, res.results), res


def kernel(**inputs):
    out, _ = _run(inputs)
    return out


# revision 66
# speedup vs baseline: 3.8833x; 1.3105x over previous
"""CentroidAttention Trainium2 kernel (8 NeuronCores, SPMD data-parallel over batch).

Reference computation (per problem):
    centers = segment_mean(features, labels, C=1000)       # [C, F]
    q = features @ Wq; k = centers @ Wk; v = centers @ Wv  # [B,A],[C,A],[C,A]
    P = softmax(q @ k.T / sqrt(A))                         # [B, C]
    attn = P @ v @ Wproj + bproj                           # [B, F]
    out = concat([features, attn], -1)                     # [B, 2F]

Sharding: batch B=16384 split 8 ways (2048 rows/core). Each core computes
partial segment sums+counts (as a one-hot matmul, transposed layout
sums.T [F, C]), AllReduce's them, then runs the attention pipeline on its
own batch shard. Weights are replicated.

Device layout choices (all matmuls are out = lhsT.T @ rhs, K on partitions):
  - sums.T [F, C]   <- lhsT = feat chunk [B,F-chunk], rhs = onehot [B, C]
  - feat.T [F, B]   <- PE transposes fused in the segsum pass (same lhsT)
  - q.T   [A, B]    <- lhsT = Wq [F, A], rhs = feat.T
  - kU.T  [A, C]    <- lhsT = Wk [F, A], rhs = sums.T      (unscaled by counts)
  - vU    [C, A]    <- lhsT = sums.T,    rhs = Wv          (unscaled)
  - S.T   [C, B]    <- lhsT = kU.T,      rhs = q.T
  - exp: ACT Exp with per-partition scale = attn_scale * recip_counts[c]
    (folds the centers division of the k-path into the softmax logits)
  - v = vU * recip_counts[c] applied on PSUM evict (folds the v-path division)
  - denom [1, B]    <- lhsT = ones[128,1], rhs = expS.T
  - attnU.T [A, B]  <- lhsT = v [C, A], rhs = expS.T   (unnormalized)
  - outU [B, F]     <- lhsT = attnU.T, rhs = Wproj, plus K=1 row
                       (lhsT=denom-row, rhs=bproj) so bias lands pre-normalized
  - out = outU * recip_denom[b]  (per-partition ACT scale on final evict)

Classes padded 1000 -> 1024 (zero one-hot columns); the padded expS.T rows are
memset to 0 so they contribute nothing to denom or PV.
"""

import numpy as np

import concourse.bass as bass
import concourse.bacc as bacc
import concourse.mybir as mybir
import concourse.tile as tile
from concourse.bass_utils import run_bass_kernel_spmd
from concourse.masks import make_identity

P = 128
B_LOCAL = 2048          # batch rows per core
F = 1024                # feature dim
A = 512                 # attention dim
C = 1000                # num classes
CP = 1024               # classes padded to a multiple of 512
NB = B_LOCAL // P       # 16 batch chunks
NF = F // P             # 8 feature chunks
NA = A // P             # 4 attn-dim chunks
NCC = CP // P           # 8 class chunks
NN = B_LOCAL // 512     # 4 moving-operand chunks over local batch
N_CORES = 8
SCALE = float(A) ** -0.5

F32 = mybir.dt.float32
BF16 = mybir.dt.bfloat16
F16 = mybir.dt.float16


def _emit(tc, collective=True, io=None):
    nc = tc.nc
    if io is None:
        io = _declare_io(nc)
    (feat_dram, lab_dram, wq_dram, wk_dram, wv_dram, wp_dram, bp_dram,
     out_dram) = io

    from contextlib import ExitStack

    with ExitStack() as ctx:
        consts = ctx.enter_context(tc.tile_pool(name="consts", bufs=1))
        stage = ctx.enter_context(tc.tile_pool(name="stage", bufs=1))
        featn_pool = ctx.enter_context(tc.tile_pool(name="featn", bufs=1))
        p1024 = ctx.enter_context(tc.tile_pool(name="p1024", bufs=1))
        t2048 = ctx.enter_context(tc.tile_pool(name="t2048", bufs=1))
        wpool = ctx.enter_context(tc.tile_pool(name="wpool", bufs=1))
        vpool = ctx.enter_context(tc.tile_pool(name="vpool", bufs=1))
        dram = ctx.enter_context(tc.tile_pool(name="dram", bufs=1, space="DRAM"))

        STAGE_BUFS = 4
        C1024_BUFS = 16
        T2048_BUFS = 12
        pf16 = ctx.enter_context(tc.tile_pool(name="pf16", bufs=1))

        def stage_tile(name):
            return stage.tile([P, 1024], F32, name=name, tag="stage", bufs=STAGE_BUFS)

        def c1024_tile(name):
            return p1024.tile([P, CP], BF16, name=name, tag="c1024", bufs=C1024_BUFS)

        def t2048_tile(name):
            return t2048.tile([P, B_LOCAL], BF16, name=name, tag="t2048",
                              bufs=T2048_BUFS)

        # ---- constants ----
        identity = consts.tile([P, P], BF16, name="identity")
        make_identity(nc, identity)
        one1 = consts.tile([1, 1], F32, name="one1")
        nc.gpsimd.memset(one1, 1.0)
        ones_col = consts.tile([P, 1], BF16, name="ones_col")
        nc.gpsimd.memset(ones_col, 1.0)
        ones_row = consts.tile([1, P], BF16, name="ones_row")
        nc.gpsimd.memset(ones_row, 1.0)
        iota_g = consts.tile([P, CP], F32, name="iota_g")
        nc.gpsimd.iota(iota_g, pattern=[[1, CP]], base=0, channel_multiplier=0,
                       allow_small_or_imprecise_dtypes=True)
        # funnel iota + labels through DVE: the one-hot tensor_scalar
        # (pointer-scalar variant) only has a single sync-wait slot
        iota = consts.tile([P, CP], F32, name="iota")
        nc.vector.tensor_copy(iota, iota_g)
        labels_ld = consts.tile([P, NB], F32, name="labels_ld")
        nc.sync.dma_start(labels_ld, lab_dram)
        labels_sb = consts.tile([P, NB], F32, name="labels_sb")
        nc.vector.tensor_copy(labels_sb, labels_ld)
        # warm the ACT Exp table during the load phase so the table DMA
        # doesn't land in the middle of the softmax
        exp_warm = consts.tile([P, 1], F32, name="exp_warm")
        nc.scalar.activation(exp_warm, labels_sb[:, 0:1],
                             mybir.ActivationFunctionType.Exp,
                             bias=0.0, scale=0.0)

        # ---- collective bounce buffers. counts ride their own tiny f32
        # reduce launched during the segsum; the two sums halves go as fp16
        # (matching the fp16 compute dtype) so each collective moves 1 MB ----
        HALF = NF // 2 * P  # 512 rows of sums.T per collective
        bcnt_in = dram.tile([1, CP], F32, name="bcnt_in")
        bcnt_out = dram.tile([1, CP], F32, name="bcnt_out",
                             addr_space="Shared")
        bounce0_in = dram.tile([HALF, CP], F16, name="bounce0_in")
        bounce0_out = dram.tile([HALF, CP], F16, name="bounce0_out",
                                addr_space="Shared")
        bounce1_in = dram.tile([HALF, CP], F16, name="bounce1_in")
        bounce1_out = dram.tile([HALF, CP], F16, name="bounce1_out",
                                addr_space="Shared")

        # ---- phase 0: load features (cast fp16) and build one-hot ----
        feats = []
        for k in range(NB):
            st = stage_tile(f"fst{k}")
            nc.sync.dma_start(st, feat_dram[k * P:(k + 1) * P, :])
            fb = featn_pool.tile([P, F], BF16, name=f"featN{k}")
            # ACT does the cast: DVE is saturated building one-hots during
            # the feature-load chase
            nc.scalar.copy(fb, st)
            feats.append(fb)
        onehots = []
        for k in range(NB):
            oh = c1024_tile(f"onehot{k}")
            nc.vector.tensor_scalar(oh, iota, labels_sb[:, k:k + 1], None,
                                    mybir.AluOpType.is_equal)
            onehots.append(oh)

        # ---- phase A: counts = ones.T @ onehot  -> bounce row F ----
        with tc.tile_pool(name="pcnt", bufs=1, space="PSUM") as pcnt:
            cps = pcnt.tile([1, CP], F32, name="counts_ps")
            for k in range(NB):
                for h in range(2):
                    nc.tensor.matmul(cps[:, h * 512:(h + 1) * 512],
                                     lhsT=ones_col,
                                     rhs=onehots[k][:, h * 512:(h + 1) * 512],
                                     start=(k == 0), stop=(k == NB - 1))
            cnt_sb = consts.tile([1, CP], F32, name="cnt_sb")
            nc.vector.tensor_copy(cnt_sb, cps)
            nc.sync.dma_start(bcnt_in, cnt_sb)
        # counts reduce in flight while the segment sums still compute
        if collective:
            nc.gpsimd.collective_compute(
                "AllReduce", mybir.AluOpType.add,
                replica_groups=[list(range(N_CORES))],
                ins=[bcnt_in.opt()], outs=[bcnt_out.opt()],
            )
        else:
            nc.sync.dma_start(bcnt_out, bcnt_in)

        # ---- phase B: segment sums (transposed) + feat.T via fused PE
        # transpose. F-chunks processed in pairs so the PE has ~2x work per
        # arriving feature chunk during the initial DMA chase.
        featTs = [None] * NF
        with tc.tile_pool(name="pseg", bufs=1, space="PSUM") as pseg:
            for jp in range(0, NF, 2):
                sps_p, ftA_p, ftB_p = {}, {}, {}
                for j in (jp, jp + 1):
                    sps_p[j] = pseg.tile([P, CP], F32, name=f"sums{j}",
                                         tag="sums", bufs=2)
                    ftA_p[j] = pseg.tile([P, F], BF16, name=f"ftA{j}",
                                         tag="ftA", bufs=2)
                    ftB_p[j] = pseg.tile([P, F], BF16, name=f"ftB{j}",
                                         tag="ftB", bufs=2)
                for k in range(NB):
                    for j in (jp, jp + 1):
                        lhsT = feats[k][:, j * P:(j + 1) * P]
                        for h in range(2):
                            nc.tensor.matmul(
                                sps_p[j][:, h * 512:(h + 1) * 512],
                                lhsT=lhsT,
                                rhs=onehots[k][:, h * 512:(h + 1) * 512],
                                start=(k == 0), stop=(k == NB - 1))
                        ft = ftA_p[j] if k < 8 else ftB_p[j]
                        nc.tensor.transpose(ft[:, (k % 8) * P:(k % 8 + 1) * P],
                                            lhsT, identity)
                for j in (jp, jp + 1):
                    ftile = t2048_tile(f"featT{j}")
                    nc.vector.tensor_copy(ftile[:, 0:F], ftA_p[j])
                    nc.vector.tensor_copy(ftile[:, F:2 * F], ftB_p[j])
                    featTs[j] = ftile
                    sums_sb = pf16.tile([P, CP], F16, name=f"sums_f16_{j}",
                                        tag="sf16", bufs=3)
                    nc.vector.tensor_copy(sums_sb, sps_p[j])
                    if j < NF // 2:
                        nc.sync.dma_start(bounce0_in[j * P:(j + 1) * P, :],
                                          sums_sb)
                    else:
                        jj = j - NF // 2
                        nc.sync.dma_start(bounce1_in[jj * P:(jj + 1) * P, :],
                                          sums_sb)
                if jp == NF // 2 - 2:
                    # first half (+counts) reduced while second half computes
                    if collective:
                        nc.gpsimd.collective_compute(
                            "AllReduce", mybir.AluOpType.add,
                            replica_groups=[list(range(N_CORES))],
                            ins=[bounce0_in.opt()], outs=[bounce0_out.opt()],
                        )
                    else:
                        nc.sync.dma_start(bounce0_out, bounce0_in)

        # ---- AllReduce second half ----
        if collective:
            nc.gpsimd.collective_compute(
                "AllReduce", mybir.AluOpType.add,
                replica_groups=[list(range(N_CORES))],
                ins=[bounce1_in.opt()], outs=[bounce1_out.opt()],
            )
        else:  # single-core timeline-sim variant: same traffic, no collective
            nc.sync.dma_start(bounce1_out, bounce1_in)

        # ---- weights: load + cast (overlaps the collective) ----
        wqb, wkb, wvb = [], [], []
        for nm, src, dst in (("wq", wq_dram, wqb), ("wk", wk_dram, wkb),
                             ("wv", wv_dram, wvb)):
            for j in range(NF):
                st = stage_tile(f"{nm}st{j}")
                nc.sync.dma_start(st[:, 0:A], src[j * P:(j + 1) * P, :])
                wb = wpool.tile([P, A], BF16, name=f"{nm}b{j}")
                nc.vector.tensor_copy(wb, st[:, 0:A])
                dst.append(wb)
        wpb = []
        for a in range(NA):
            st = stage_tile(f"wpst{a}")
            nc.sync.dma_start(st, wp_dram[a * P:(a + 1) * P, :])
            wb = wpool.tile([P, F], BF16, name=f"wpb{a}")
            nc.vector.tensor_copy(wb, st)
            wpb.append(wb)
        bst = stage_tile("bst")
        nc.sync.dma_start(bst[0:1, :], bp_dram)
        bprojb = wpool.tile([1, F], BF16, name="bprojb")
        nc.vector.tensor_copy(bprojb, bst[0:1, :])

        # ---- q.T = Wq.T @ feat.T (PE busy during the collective) ----
        qTs = []
        with tc.tile_pool(name="pq", bufs=1, space="PSUM") as pq:
            for a in range(NA):
                qt = t2048_tile(f"qT{a}")
                for nh in range(2):
                    qps = pq.tile([P, F], F32, name=f"qps{a}_{nh}",
                                  tag="q", bufs=4)
                    for j in range(NF):
                        for n in range(2):
                            nc.tensor.matmul(qps[:, n * 512:(n + 1) * 512],
                                             lhsT=wqb[j][:, a * P:(a + 1) * P],
                                             rhs=featTs[j][:, (nh * 2 + n) * 512:
                                                           (nh * 2 + n + 1) * 512],
                                             start=(j == 0), stop=(j == NF - 1))
                    nc.scalar.copy(qt[:, nh * F:(nh + 1) * F], qps)
                qTs.append(qt)

        # ---- read back reduced sums + counts (fp16 lands matmul-ready) ----
        sumsb = []
        for j in range(NF):
            sb = c1024_tile(f"sumsb{j}")
            if j < NF // 2:
                nc.sync.dma_start(sb, bounce0_out[j * P:(j + 1) * P, :])
            else:
                jj = j - NF // 2
                nc.sync.dma_start(sb, bounce1_out[jj * P:(jj + 1) * P, :])
            sumsb.append(sb)
        counts_sb = consts.tile([1, CP], F32, name="counts_sb")
        nc.sync.dma_start(counts_sb, bcnt_out)

        kTs, vbs = [], []
        with tc.tile_pool(name="pkv", bufs=1, space="PSUM") as pkv:
            # recip_counts in [C-chunk(partition), chunk-idx] layout
            cpsT = pkv.tile([P, NCC], F32, name="cntT")
            for c in range(NCC):
                nc.tensor.transpose(cpsT[:, c:c + 1],
                                    counts_sb[:, c * P:(c + 1) * P], one1)
            cnt_m = consts.tile([P, NCC], F32, name="cnt_m")
            nc.vector.tensor_scalar_max(cnt_m, cpsT, 1.0)
            recip_cols = consts.tile([P, NCC], F32, name="recip_cols")
            nc.vector.reciprocal(recip_cols, cnt_m)
            exp_scale = consts.tile([P, NCC], F32, name="exp_scale")
            nc.vector.tensor_scalar_mul(exp_scale, recip_cols, SCALE)

            # kU.T [A, C] ; counts division folded into the exp scale later
            for a in range(NA):
                kps = pkv.tile([P, CP], F32, name=f"kps{a}", tag="k", bufs=2)
                for j in range(NF):
                    for h in range(2):
                        nc.tensor.matmul(kps[:, h * 512:(h + 1) * 512],
                                         lhsT=wkb[j][:, a * P:(a + 1) * P],
                                         rhs=sumsb[j][:, h * 512:(h + 1) * 512],
                                         start=(j == 0), stop=(j == NF - 1))
                kt = c1024_tile(f"kT{a}")
                nc.scalar.copy(kt, kps)
                kTs.append(kt)

            # v [C, A] = (sums.T).T @ Wv, scaled by recip_counts on evict
            for c in range(NCC):
                vps = pkv.tile([P, A], F32, name=f"vps{c}", tag="v", bufs=2)
                for j in range(NF):
                    nc.tensor.matmul(vps,
                                     lhsT=sumsb[j][:, c * P:(c + 1) * P],
                                     rhs=wvb[j],
                                     start=(j == 0), stop=(j == NF - 1))
                vb = vpool.tile([P, A], BF16, name=f"vb{c}")
                nc.scalar.activation(vb, vps,
                                     mybir.ActivationFunctionType.Copy,
                                     bias=0.0, scale=recip_cols[:, c:c + 1])
                vbs.append(vb)

        # ---- S.T [C, B] and exp (centers division folded into scale) ----
        expSTs = []
        with tc.tile_pool(name="pst", bufs=1, space="PSUM") as pst:
            for c in range(NCC):
                est = t2048_tile(f"expST{c}")
                rows = (C - c * P) if c == NCC - 1 else P
                if rows < P:
                    # zero the padded class rows; exp overwrites the valid ones
                    nc.vector.memset(est, 0.0)
                for nh in range(2):
                    sps = pst.tile([P, F], F32, name=f"stps{c}_{nh}",
                                   tag="st", bufs=4)
                    for a in range(NA):
                        for n in range(2):
                            nc.tensor.matmul(
                                sps[:, n * 512:(n + 1) * 512],
                                lhsT=kTs[a][:, c * P:(c + 1) * P],
                                rhs=qTs[a][:, (nh * 2 + n) * 512:
                                            (nh * 2 + n + 1) * 512],
                                start=(a == 0), stop=(a == NA - 1))
                    nc.scalar.activation(est[0:rows, nh * F:(nh + 1) * F],
                                         sps[0:rows, :],
                                         mybir.ActivationFunctionType.Exp,
                                         bias=0.0,
                                         scale=exp_scale[0:rows, c:c + 1])
                expSTs.append(est)

        # ---- attnU.T [A, B] = v.T @ expS.T (unnormalized), with the softmax
        # denominator accumulated in the same phase (a==0 pass) ----
        recipD_cols = consts.tile([P, NB], F32, name="recipD_cols")
        recipD = consts.tile([1, B_LOCAL], F32, name="recipD")
        attnTs = []
        with tc.tile_pool(name="ppv", bufs=1, space="PSUM") as ppv:
            dps = ppv.tile([1, B_LOCAL], F32, name="dps")
            for a in range(NA):
                at = t2048_tile(f"attnT{a}")
                for nh in range(2):
                    aps = ppv.tile([P, F], F32, name=f"aps{a}_{nh}",
                                   tag="av", bufs=2)
                    for c in range(NCC):
                        for n in range(2):
                            nc.tensor.matmul(
                                aps[:, n * 512:(n + 1) * 512],
                                lhsT=vbs[c][:, a * P:(a + 1) * P],
                                rhs=expSTs[c][:, (nh * 2 + n) * 512:
                                              (nh * 2 + n + 1) * 512],
                                start=(c == 0), stop=(c == NCC - 1))
                        if a == 0:
                            for n in range(2):
                                nc.tensor.matmul(
                                    dps[:, (nh * 2 + n) * 512:
                                        (nh * 2 + n + 1) * 512],
                                    lhsT=ones_col,
                                    rhs=expSTs[c][:, (nh * 2 + n) * 512:
                                                  (nh * 2 + n + 1) * 512],
                                    start=(c == 0), stop=(c == NCC - 1))
                    evict = nc.vector.tensor_copy if nh == 0 else nc.scalar.copy
                    evict(at[:, nh * F:(nh + 1) * F], aps)
                if a == 0:
                    nc.vector.reciprocal(recipD, dps)
                attnTs.append(at)

        # ---- out = (attnU.T.T @ Wproj) * recip_denom + bproj ----
        with tc.tile_pool(name="po", bufs=1, space="PSUM") as po:
            rdps = po.tile([P, NB], F32, name="rdps")
            for t in range(NB):
                nc.tensor.transpose(rdps[:, t:t + 1],
                                    recipD[:, t * P:(t + 1) * P], one1)
            nc.vector.tensor_copy(recipD_cols, rdps)
            # bproj broadcast to all partitions (bias applied post-normalize)
            bpb_ps = po.tile([P, F], F32, name="bpb_ps")
            for h in range(2):
                nc.tensor.matmul(bpb_ps[:, h * 512:(h + 1) * 512],
                                 lhsT=ones_row,
                                 rhs=bprojb[:, h * 512:(h + 1) * 512],
                                 start=True, stop=True)
            bpb_sb = consts.tile([P, F], F32, name="bpb_sb")
            nc.vector.tensor_copy(bpb_sb, bpb_ps)
            for t in range(NB):
                ops = po.tile([P, F], F32, name=f"ops{t}", tag="o", bufs=2)
                for a in range(NA):
                    for h in range(2):
                        nc.tensor.matmul(ops[:, h * 512:(h + 1) * 512],
                                         lhsT=attnTs[a][:, t * P:(t + 1) * P],
                                         rhs=wpb[a][:, h * 512:(h + 1) * 512],
                                         start=(a == 0), stop=(a == NA - 1))
                osb = stage_tile(f"osb{t}")
                nc.vector.scalar_tensor_tensor(
                    osb, ops, recipD_cols[:, t:t + 1], bpb_sb,
                    op0=mybir.AluOpType.mult, op1=mybir.AluOpType.add)
                nc.sync.dma_start(out_dram[t * P:(t + 1) * P, :], osb)


def _declare_io(nc):
    return (
        nc.dram_tensor("features", [B_LOCAL, F], F32, kind="ExternalInput")[:],
        nc.dram_tensor("labels_f32", [P, NB], F32, kind="ExternalInput")[:],
        nc.dram_tensor("Wq", [F, A], F32, kind="ExternalInput")[:],
        nc.dram_tensor("Wk", [F, A], F32, kind="ExternalInput")[:],
        nc.dram_tensor("Wv", [F, A], F32, kind="ExternalInput")[:],
        nc.dram_tensor("Wproj", [A, F], F32, kind="ExternalInput")[:],
        nc.dram_tensor("bproj", [1, F], F32, kind="ExternalInput")[:],
        nc.dram_tensor("out", [B_LOCAL, F], F32, kind="ExternalOutput")[:],
    )


_BUILT = {}


def _get_nc(collective=True, reps=1):
    key = (collective, reps)
    if key not in _BUILT:
        nc = bacc.Bacc("TRN2", target_bir_lowering=False, debug=False,
                       num_devices=N_CORES)
        with tile.TileContext(nc) as tc:
            io = _declare_io(nc)
            for r in range(reps):
                if r:
                    tc.strict_bb_all_engine_barrier()
                _emit(tc, collective=collective, io=io)
        nc.compile()
        _BUILT[key] = nc
    return _BUILT[key]


def _make_in_maps(inputs):
    features = np.ascontiguousarray(np.asarray(inputs["features"],
                                               dtype=np.float32))
    labels = np.ascontiguousarray(np.asarray(inputs["labels"])).astype(np.int64)
    Wq = np.ascontiguousarray(np.asarray(inputs["Wq"], dtype=np.float32))
    Wk = np.ascontiguousarray(np.asarray(inputs["Wk"], dtype=np.float32))
    Wv = np.ascontiguousarray(np.asarray(inputs["Wv"], dtype=np.float32))
    Wproj = np.ascontiguousarray(np.asarray(inputs["Wproj"], dtype=np.float32))
    bproj = np.ascontiguousarray(
        np.asarray(inputs["bproj"], dtype=np.float32)).reshape(1, F)

    in_maps = []
    for cix in range(N_CORES):
        fl = features[cix * B_LOCAL:(cix + 1) * B_LOCAL]
        ll = labels[cix * B_LOCAL:(cix + 1) * B_LOCAL]
        lab2d = np.ascontiguousarray(
            ll.astype(np.float32).reshape(NB, P).T)
        in_maps.append({
            "features": fl,
            "labels_f32": lab2d,
            "Wq": Wq, "Wk": Wk, "Wv": Wv, "Wproj": Wproj, "bproj": bproj,
        })
    return in_maps


def _assemble(inputs, results):
    features = np.asarray(inputs["features"], dtype=np.float32)
    out = np.empty((N_CORES * B_LOCAL, 2 * F), np.float32)
    out[:, :F] = features
    for cix in range(N_CORES):
        out[cix * B_LOCAL:(cix + 1) * B_LOCAL, F:] = results[cix]["out"]
    return out


def _run(inputs, **run_kwargs):
    nc = _get_nc()
    in_maps = _make_in_maps(inputs)
    res = run_bass_kernel_spmd(nc, in_maps, list(range(N_CORES)), **run_kwargs)
    return _assemble(inputs, res.results), res


def kernel(**inputs):
    out, _ = _run(inputs)
    return out
